# revision 22
# baseline (speedup 1.0000x reference)
"""Trainium2 Bass kernel for nn_ContourIntegrationLayer.

Reference computes a depthwise 25x25 conv with a *masked* kernel:
only channels 5 (horizontal), 10 (vertical), 54 & 67 (diagonal) have
any nonzero taps -- 8 taps each at offsets +-{3,6,9,12}. Every other
channel reduces to out = x + bias[c]. The full op is
    out = y * x + bias + x        (y = masked depthwise conv of x)

Strategy (per core, batch-parallel over 8 cores, 8 images/core):
  Host permutes the 768 (b,c)-images so the 32 "special" ones (the 4
  conv channels x 8 batches) are the LAST 32 rows -> phase A and
  phase B touch disjoint DRAM rows: no write-after-write ordering.

  Phase A: rows 0..735 streamed as [128, 6272] tiles; one
           tensor_scalar_add per tile (per-partition bias operand),
           output written in fp16 (halves store traffic; tolerance is
           2e-2, fp16 costs ~2e-4 in rel-l2).
  Phase B: the 32 special images in (112,112) layout; each stencil
           tap is a TensorE matmul (fp16 weights/ifmap, fp32 psum)
           with a host-built shifted/banded 112x112 matrix.  PSUM
           holds y; VectorE computes (y+1)*x straight out of PSUM,
           GpSimdE adds the channel bias, fp16 result stored per
           4-image batch.  All woven into the phase-A chunk loop so
           it hides under the HBM stream.
  Host casts the fp16 result back to fp32 and un-permutes.
"""

import numpy as np

# ---- problem constants (hardcoded; kernel.py must be self-contained) ----
B_FULL = 64
CH = 96
H = W = 112
HW = H * W
N_CORES = 8
B_SHARD = B_FULL // N_CORES          # 8 images per core
N_IMG = B_SHARD * CH                 # 768 (b,c)-images per core
SPECIALS = (5, 10, 54, 67)
N_SPEC = B_SHARD * len(SPECIALS)     # 32 special images per core
N_MAIN = N_IMG - N_SPEC              # 736 plain rows
CHUNK = HW // 2                      # 6272 free-dim chunk (1.6MB fp16 tiles)
NKT = (N_MAIN + 127) // 128          # 6 partition tiles (last has 96 rows)
IDX = (0, 3, 6, 9, 15, 18, 21, 24)   # masked kernel tap positions
OFFS = tuple(i - 12 for i in IDX)    # spatial offsets: +-{3,6,9,12}
NMAT = 25                            # banded-v, 8 diag(ch5), 8+8 banded-diag

# host-side row permutation (same for every shard): plain rows first,
# then the specials in (batch-major, channel 5/10/54/67) order
_MAIN_ROWS = [r for r in range(N_IMG) if (r % CH) not in SPECIALS]
_SPEC_ROWS = [b * CH + c for b in range(B_SHARD) for c in SPECIALS]
PERM = np.array(_MAIN_ROWS + _SPEC_ROWS, dtype=np.int64)

TRACE = False
LAST_EXEC_NS = None


def _build_program():
    import concourse.bacc as bacc
    import concourse.mybir as mybir
    from concourse.tile import TileContext

    f32 = mybir.dt.float32
    f16 = mybir.dt.float16
    alu = mybir.AluOpType
    # Bacc (not plain Bass): its compile() pipeline splits multi-wait
    # instructions into EventSemaphores (TRN2 allows 1 wait/instruction)
    nc = bacc.Bacc("TRN2")
    x3 = nc.dram_tensor("x", [N_MAIN, H, W], f16, kind="ExternalInput")
    # special images, host-pretransposed to (h, j*w): plain 2D loads/stores
    xs_d = nc.dram_tensor("xs", [H, N_SPEC * W], f16, kind="ExternalInput")
    mats = nc.dram_tensor("mats", [H, NMAT * W], f16, kind="ExternalInput")
    biast = nc.dram_tensor("biast", [128, NKT + 4], f32, kind="ExternalInput")
    out3 = nc.dram_tensor("out", [N_MAIN, H, W], f16, kind="ExternalOutput")
    outs_d = nc.dram_tensor("outs", [H, N_SPEC * W], f16, kind="ExternalOutput")

    # per-channel tap list: (matrix block index, column offset)
    taps = {
        5: [(1 + t, OFFS[t]) for t in range(8)],
        10: [(0, 0)],
        54: [(9 + t, OFFS[t]) for t in range(8)],
        67: [(17 + t, OFFS[t]) for t in range(8)],
    }

    # chunk iteration order: (row0, nrows, col0, width); the last k-tile
    # runs in quarter-width chunks so the drain tail is short
    chunks = []
    for k in range(NKT):
        r0 = k * 128
        p = min(128, N_MAIN - r0)
        if k < NKT - 1:
            for ci in range(2):
                chunks.append((r0, p, ci * CHUNK, CHUNK))
        else:
            for ci in range(4):
                chunks.append((r0, p, ci * (CHUNK // 2), CHUNK // 2))

    with TileContext(nc) as tc:
        with (
            tc.tile_pool(name="const", bufs=1) as cpool,
            tc.tile_pool(name="pa_in", bufs=6) as pin_pool,
            tc.tile_pool(name="pa_out", bufs=8) as pout_pool,
            tc.tile_pool(name="pb_out", bufs=3) as pbo_pool,
            tc.tile_pool(name="pb_tmp", bufs=6) as pbt_pool,
            tc.tile_pool(name="psum", bufs=8, space="PSUM") as psum_pool,
        ):
            # bias first on the sync ring (tiny); mats and the special
            # images are woven between early chunk loads below so load 0
            # starts almost immediately and no issue bubble exceeds the
            # sync engine's per-iteration slack
            # consts ride the scalar ring: ~1.5MB clears it by ~6us,
            # well before the first bulk store (~14us), and the sync ring
            # issues chunk load 0 at t~0 with zero preamble
            bias_sb = cpool.tile([128, NKT + 4], f32)
            nc.scalar.dma_start(out=bias_sb[:], in_=biast[:, :])
            mats_sb = cpool.tile([H, NMAT * W], f16)
            nc.scalar.dma_start(out=mats_sb[:], in_=mats[:, :])
            xs_all = cpool.tile([H, N_SPEC * W], f16)
            nc.scalar.dma_start(out=xs_all[:], in_=xs_d[:, :])

            xf = x3[:, :, :].rearrange("n h w -> n (h w)")
            of = out3[:, :, :].rearrange("n h w -> n (h w)")

            def emit_matmuls(b):
                ps_tiles = []
                for si, c in enumerate(SPECIALS):
                    j = b * 4 + si
                    ps = psum_pool.tile([H, W], f32, tag="ps")
                    tl = taps[c]
                    for i, (mi, co) in enumerate(tl):
                        a = max(co, 0)
                        bb = W + min(co, 0)
                        nc.tensor.matmul(
                            ps[:, a - co:bb - co],
                            mats_sb[:, mi * W:(mi + 1) * W],
                            xs_all[:, j * W + a:j * W + bb],
                            start=(i == 0),
                            stop=(i == len(tl) - 1),
                        )
                    ps_tiles.append(ps)
                return ps_tiles

            def emit_finish(b, ps_tiles):
                ob4 = pbo_pool.tile([H, 4 * W], f16, tag="pbo")
                for si in range(4):
                    j = b * 4 + si
                    # tmp = (y + 1) * x   (PSUM read on VectorE, fp16 out)
                    tmp = pbt_pool.tile([H, W], f32, tag="pst")
                    nc.vector.scalar_tensor_tensor(
                        out=tmp[:],
                        in0=ps_tiles[si][:],
                        scalar=1.0,
                        in1=xs_all[:, j * W:(j + 1) * W],
                        op0=alu.add,
                        op1=alu.mult,
                    )
                    # out = tmp + bias[c]  (VectorE, no cross-engine wait)
                    nc.vector.tensor_scalar_add(
                        out=ob4[:, si * W:(si + 1) * W],
                        in0=tmp[:],
                        scalar1=bias_sb[:H, NKT + si:NKT + si + 1],
                    )
                # one store per 4-image batch, on the otherwise-idle
                # gpsimd ring: never head-of-line-blocks the bulk stores
                nc.gpsimd.dma_start(
                    out=outs_d[:, 4 * b * W:(4 * b + 4) * W],
                    in_=ob4[:],
                )

            in_flight = []
            for it, (r0, p, c0, w) in enumerate(chunks):
                k = r0 // 128
                tin = pin_pool.tile([128, CHUNK], f16, tag="pin")
                nc.sync.dma_start(
                    out=tin[:p, :w],
                    in_=xf[r0:r0 + p, c0:c0 + w],
                )
                tout = pout_pool.tile([128, CHUNK], f16, tag="pout")
                nc.vector.tensor_scalar_add(
                    out=tout[:p, :w], in0=tin[:p, :w],
                    scalar1=bias_sb[:p, k:k + 1],
                )
                nc.scalar.dma_start(
                    out=of[r0:r0 + p, c0:c0 + w],
                    in_=tout[:p, :w],
                )


                # phase B, software-pipelined: batch b's matmuls start at
                # iteration b+1, its PSUM drain at b+3 -- TensorE gets a
                # ~2-iteration head start so VectorE never waits on it
                if 3 <= it < 3 + B_SHARD:
                    emit_finish(*in_flight.pop(0))
                if 1 <= it < 1 + B_SHARD:
                    in_flight.append((it - 1, emit_matmuls(it - 1)))
            while in_flight:
                emit_finish(*in_flight.pop(0))

    if not nc.is_finalized():
        nc.finalize()  # runs Bacc.compile(): reg alloc + wait splitting
    return nc


def _build_host_consts(raw_kernel, bias):
    rk = np.asarray(raw_kernel, dtype=np.float32)
    bz = np.asarray(bias, dtype=np.float32).reshape(CH)
    idx = np.array(IDX)
    w5 = rk[5, 12, idx]
    w10 = rk[10, idx, 12]
    w54 = rk[54, idx, idx]
    w67 = rk[67, idx, idx]

    blocks = np.zeros((NMAT, H, H), np.float32)
    for t, d in enumerate(OFFS):
        # row-shift matrix: lhsT[i, j] = w * delta(i == j + d)
        blocks[0] += w10[t] * np.eye(H, k=-d, dtype=np.float32)
        blocks[1 + t] = w5[t] * np.eye(H, dtype=np.float32)
        blocks[9 + t] = w54[t] * np.eye(H, k=-d, dtype=np.float32)
        blocks[17 + t] = w67[t] * np.eye(H, k=-d, dtype=np.float32)

    mats_host = np.ascontiguousarray(
        blocks.transpose(1, 0, 2).reshape(H, NMAT * H).astype(np.float16)
    )
    # per-partition bias for the permuted plain rows, one column per k-tile,
    # plus 4 replicated columns for the special channels
    biast_host = np.zeros((128, NKT + 4), np.float32)
    for i, r in enumerate(_MAIN_ROWS):
        biast_host[i % 128, i // 128] = bz[r % CH]
    for si, c in enumerate(SPECIALS):
        biast_host[:, NKT + si] = bz[c]
    return mats_host, biast_host


_PROGRAM = None


def kernel(x, raw_kernel, bias):
    global _PROGRAM, LAST_EXEC_NS
    from concourse.bass_utils import run_bass_kernel_spmd

    x16 = np.asarray(x).astype(np.float16)
    mats_host, biast_host = _build_host_consts(raw_kernel, bias)

    if _PROGRAM is None:
        _PROGRAM = _build_program()
    nc = _PROGRAM

    in_maps = []
    for s in range(N_CORES):
        shard = x16[s * B_SHARD:(s + 1) * B_SHARD].reshape(N_IMG, H, W)
        main = np.ascontiguousarray(shard[PERM[:N_MAIN]])
        xs_host = np.ascontiguousarray(
            shard[PERM[N_MAIN:]].transpose(1, 0, 2).reshape(H, N_SPEC * W)
        )
        in_maps.append(
            {"x": main, "xs": xs_host, "mats": mats_host, "biast": biast_host}
        )

    res = None
    if TRACE:
        # DIY NTFF capture: the container's antenv lacks axon_hooks, so
        # bass_utils' trace path can't run; drive the .so hook directly.
        try:
            import os

            from trn_agent_boot.trn_boot import _ntff_profile_via_ctypes

            hook_factory = _ntff_profile_via_ctypes("/opt/axon/libaxon_pjrt.so")
            prof_dir = os.environ.get("KPROF_DIR", os.path.abspath("./prof"))
            os.makedirs(prof_dir, exist_ok=True)
            with hook_factory(prof_dir, [0]):
                res = run_bass_kernel_spmd(
                    nc, in_maps, core_ids=list(range(N_CORES))
                )
        except Exception as e:  # noqa: BLE001
            print("profiling failed, running untraced:", e)
            res = None
    if res is None:
        res = run_bass_kernel_spmd(nc, in_maps, core_ids=list(range(N_CORES)))
    LAST_EXEC_NS = res.exec_time_ns

    out = np.empty((B_FULL, CH, H, W), dtype=np.float32)
    for s in range(N_CORES):
        shard_view = out[s * B_SHARD:(s + 1) * B_SHARD].reshape(N_IMG, H, W)
        shard_view[PERM[:N_MAIN]] = res.results[s]["out"].astype(np.float32)
        shard_view[PERM[N_MAIN:]] = (
            res.results[s]["outs"]
            .reshape(H, N_SPEC, W)
            .transpose(1, 0, 2)
            .astype(np.float32)
        )
    return out


# revision 23
# speedup vs baseline: 1.0799x; 1.0799x over previous
"""Trainium2 Bass kernel for nn_ContourIntegrationLayer.

Reference computes a depthwise 25x25 conv with a *masked* kernel:
only channels 5 (horizontal), 10 (vertical), 54 & 67 (diagonal) have
any nonzero taps -- 8 taps each at offsets +-{3,6,9,12}. Every other
channel reduces to out = x + bias[c]. The full op is
    out = y * x + bias + x        (y = masked depthwise conv of x)

Strategy (per core, batch-parallel over 8 cores, 8 images/core):
  Host permutes the 768 (b,c)-images so the 32 "special" ones (the 4
  conv channels x 8 batches) are the LAST 32 rows -> phase A and
  phase B touch disjoint DRAM rows: no write-after-write ordering.

  Phase A: rows 0..735 streamed as [128, 6272] tiles; one
           tensor_scalar_add per tile (per-partition bias operand),
           output written in fp16 (halves store traffic; tolerance is
           2e-2, fp16 costs ~2e-4 in rel-l2).
  Phase B: the 32 special images in (112,112) layout; each stencil
           tap is a TensorE matmul (fp16 weights/ifmap, fp32 psum)
           with a host-built shifted/banded 112x112 matrix.  PSUM
           holds y; VectorE computes (y+1)*x straight out of PSUM,
           GpSimdE adds the channel bias, fp16 result stored per
           4-image batch.  All woven into the phase-A chunk loop so
           it hides under the HBM stream.
  Host casts the fp16 result back to fp32 and un-permutes.
"""

import numpy as np

# ---- problem constants (hardcoded; kernel.py must be self-contained) ----
B_FULL = 64
CH = 96
H = W = 112
HW = H * W
N_CORES = 8
B_SHARD = B_FULL // N_CORES          # 8 images per core
N_IMG = B_SHARD * CH                 # 768 (b,c)-images per core
SPECIALS = (5, 10, 54, 67)
N_SPEC = B_SHARD * len(SPECIALS)     # 32 special images per core
N_MAIN = N_IMG - N_SPEC              # 736 plain rows
CHUNK = HW // 2                      # 6272 free-dim chunk (1.6MB fp16 tiles)
NKT = (N_MAIN + 127) // 128          # 6 partition tiles (last has 96 rows)
IDX = (0, 3, 6, 9, 15, 18, 21, 24)   # masked kernel tap positions
OFFS = tuple(i - 12 for i in IDX)    # spatial offsets: +-{3,6,9,12}
NMAT = 25                            # banded-v, 8 diag(ch5), 8+8 banded-diag

# host-side row permutation (same for every shard): plain rows first,
# then the specials in (batch-major, channel 5/10/54/67) order
_MAIN_ROWS = [r for r in range(N_IMG) if (r % CH) not in SPECIALS]
_SPEC_ROWS = [b * CH + c for b in range(B_SHARD) for c in SPECIALS]
PERM = np.array(_MAIN_ROWS + _SPEC_ROWS, dtype=np.int64)

TRACE = False
LAST_EXEC_NS = None


def _build_program():
    import concourse.bacc as bacc
    import concourse.mybir as mybir
    from concourse.tile import TileContext

    f32 = mybir.dt.float32
    f16 = mybir.dt.float16
    alu = mybir.AluOpType
    # Bacc (not plain Bass): its compile() pipeline splits multi-wait
    # instructions into EventSemaphores (TRN2 allows 1 wait/instruction)
    nc = bacc.Bacc("TRN2")
    x3 = nc.dram_tensor("x", [N_MAIN, H, W], f16, kind="ExternalInput")
    # special images, host-pretransposed to (h, j*w): plain 2D loads/stores
    xs_d = nc.dram_tensor("xs", [H, N_SPEC * W], f16, kind="ExternalInput")
    mats = nc.dram_tensor("mats", [H, NMAT * W], f16, kind="ExternalInput")
    biast = nc.dram_tensor("biast", [128, NKT + 4], f32, kind="ExternalInput")
    out3 = nc.dram_tensor("out", [N_MAIN, H, W], f16, kind="ExternalOutput")
    outs_d = nc.dram_tensor("outs", [H, N_SPEC * W], f16, kind="ExternalOutput")

    # per-channel tap list: (matrix block index, column offset)
    taps = {
        5: [(1 + t, OFFS[t]) for t in range(8)],
        10: [(0, 0)],
        54: [(9 + t, OFFS[t]) for t in range(8)],
        67: [(17 + t, OFFS[t]) for t in range(8)],
    }

    # chunk iteration order: (row0, nrows, col0, width); the last k-tile
    # runs in quarter-width chunks so the drain tail is short
    chunks = []
    for k in range(NKT):
        r0 = k * 128
        p = min(128, N_MAIN - r0)
        if k < NKT - 1:
            for ci in range(2):
                chunks.append((r0, p, ci * CHUNK, CHUNK))
        else:
            for ci in range(4):
                chunks.append((r0, p, ci * (CHUNK // 2), CHUNK // 2))

    with TileContext(nc) as tc:
        with (
            tc.tile_pool(name="const", bufs=1) as cpool,
            tc.tile_pool(name="pa_in", bufs=6) as pin_pool,
            tc.tile_pool(name="pa_out", bufs=7) as pout_pool,
            tc.tile_pool(name="pb_out", bufs=3) as pbo_pool,
            tc.tile_pool(name="pb_tmp", bufs=6) as pbt_pool,
            tc.tile_pool(name="psum", bufs=8, space="PSUM") as psum_pool,
        ):
            # bias first on the sync ring (tiny); mats and the special
            # images are woven between early chunk loads below so load 0
            # starts almost immediately and no issue bubble exceeds the
            # sync engine's per-iteration slack
            # consts ride the scalar ring: ~1.5MB clears it by ~6us,
            # well before the first bulk store (~14us), and the sync ring
            # issues chunk load 0 at t~0 with zero preamble
            bias_sb = cpool.tile([128, NKT + 4], f32)
            nc.scalar.dma_start(out=bias_sb[:], in_=biast[:, :])
            mats_sb = cpool.tile([H, NMAT * W], f16)
            nc.scalar.dma_start(out=mats_sb[:], in_=mats[:, :])
            xs_all = cpool.tile([H, N_SPEC * W], f16)
            nc.scalar.dma_start(out=xs_all[:], in_=xs_d[:, :])

            xf = x3[:, :, :].rearrange("n h w -> n (h w)")
            of = out3[:, :, :].rearrange("n h w -> n (h w)")

            def emit_matmuls(b):
                ps_tiles = []
                for si, c in enumerate(SPECIALS):
                    j = b * 4 + si
                    ps = psum_pool.tile([H, W], f32, tag="ps")
                    tl = taps[c]
                    for i, (mi, co) in enumerate(tl):
                        a = max(co, 0)
                        bb = W + min(co, 0)
                        nc.tensor.matmul(
                            ps[:, a - co:bb - co],
                            mats_sb[:, mi * W:(mi + 1) * W],
                            xs_all[:, j * W + a:j * W + bb],
                            start=(i == 0),
                            stop=(i == len(tl) - 1),
                        )
                    ps_tiles.append(ps)
                return ps_tiles

            def emit_finish(b, ps_tiles):
                ob4 = pbo_pool.tile([H, 4 * W], f16, tag="pbo")
                for si in range(4):
                    j = b * 4 + si
                    # tmp = (y + 1) * x   (PSUM read on VectorE, fp16 out)
                    tmp = pbt_pool.tile([H, W], f32, tag="pst")
                    nc.vector.scalar_tensor_tensor(
                        out=tmp[:],
                        in0=ps_tiles[si][:],
                        scalar=1.0,
                        in1=xs_all[:, j * W:(j + 1) * W],
                        op0=alu.add,
                        op1=alu.mult,
                    )
                    # out = tmp + bias[c]  (VectorE, no cross-engine wait)
                    nc.vector.tensor_scalar_add(
                        out=ob4[:, si * W:(si + 1) * W],
                        in0=tmp[:],
                        scalar1=bias_sb[:H, NKT + si:NKT + si + 1],
                    )
                # one store per 4-image batch, on the otherwise-idle
                # gpsimd ring: never head-of-line-blocks the bulk stores
                nc.gpsimd.dma_start(
                    out=outs_d[:, 4 * b * W:(4 * b + 4) * W],
                    in_=ob4[:],
                )

            in_flight = []
            for it, (r0, p, c0, w) in enumerate(chunks):
                k = r0 // 128
                tin = pin_pool.tile([128, CHUNK], f16, tag="pin")
                nc.sync.dma_start(
                    out=tin[:p, :w],
                    in_=xf[r0:r0 + p, c0:c0 + w],
                )
                tout = pout_pool.tile([128, CHUNK], f16, tag="pout")
                nc.vector.tensor_scalar_add(
                    out=tout[:p, :w], in0=tin[:p, :w],
                    scalar1=bias_sb[:p, k:k + 1],
                )
                nc.scalar.dma_start(
                    out=of[r0:r0 + p, c0:c0 + w],
                    in_=tout[:p, :w],
                )


                # phase B, software-pipelined: batch b's matmuls start at
                # iteration b+1, its PSUM drain at b+3 -- TensorE gets a
                # ~2-iteration head start so VectorE never waits on it
                if 3 <= it < 3 + B_SHARD:
                    emit_finish(*in_flight.pop(0))
                if 1 <= it < 1 + B_SHARD:
                    in_flight.append((it - 1, emit_matmuls(it - 1)))
            while in_flight:
                emit_finish(*in_flight.pop(0))

    if not nc.is_finalized():
        nc.finalize()  # runs Bacc.compile(): reg alloc + wait splitting
    return nc


def _build_host_consts(raw_kernel, bias):
    rk = np.asarray(raw_kernel, dtype=np.float32)
    bz = np.asarray(bias, dtype=np.float32).reshape(CH)
    idx = np.array(IDX)
    w5 = rk[5, 12, idx]
    w10 = rk[10, idx, 12]
    w54 = rk[54, idx, idx]
    w67 = rk[67, idx, idx]

    blocks = np.zeros((NMAT, H, H), np.float32)
    for t, d in enumerate(OFFS):
        # row-shift matrix: lhsT[i, j] = w * delta(i == j + d)
        blocks[0] += w10[t] * np.eye(H, k=-d, dtype=np.float32)
        blocks[1 + t] = w5[t] * np.eye(H, dtype=np.float32)
        blocks[9 + t] = w54[t] * np.eye(H, k=-d, dtype=np.float32)
        blocks[17 + t] = w67[t] * np.eye(H, k=-d, dtype=np.float32)

    mats_host = np.ascontiguousarray(
        blocks.transpose(1, 0, 2).reshape(H, NMAT * H).astype(np.float16)
    )
    # per-partition bias for the permuted plain rows, one column per k-tile,
    # plus 4 replicated columns for the special channels
    biast_host = np.zeros((128, NKT + 4), np.float32)
    for i, r in enumerate(_MAIN_ROWS):
        biast_host[i % 128, i // 128] = bz[r % CH]
    for si, c in enumerate(SPECIALS):
        biast_host[:, NKT + si] = bz[c]
    return mats_host, biast_host


_PROGRAM = None


def kernel(x, raw_kernel, bias):
    global _PROGRAM, LAST_EXEC_NS
    from concourse.bass_utils import run_bass_kernel_spmd

    x16 = np.asarray(x).astype(np.float16)
    mats_host, biast_host = _build_host_consts(raw_kernel, bias)

    if _PROGRAM is None:
        _PROGRAM = _build_program()
    nc = _PROGRAM

    in_maps = []
    for s in range(N_CORES):
        shard = x16[s * B_SHARD:(s + 1) * B_SHARD].reshape(N_IMG, H, W)
        main = np.ascontiguousarray(shard[PERM[:N_MAIN]])
        xs_host = np.ascontiguousarray(
            shard[PERM[N_MAIN:]].transpose(1, 0, 2).reshape(H, N_SPEC * W)
        )
        in_maps.append(
            {"x": main, "xs": xs_host, "mats": mats_host, "biast": biast_host}
        )

    res = None
    if TRACE:
        # DIY NTFF capture: the container's antenv lacks axon_hooks, so
        # bass_utils' trace path can't run; drive the .so hook directly.
        try:
            import os

            from trn_agent_boot.trn_boot import _ntff_profile_via_ctypes

            hook_factory = _ntff_profile_via_ctypes("/opt/axon/libaxon_pjrt.so")
            prof_dir = os.environ.get("KPROF_DIR", os.path.abspath("./prof"))
            os.makedirs(prof_dir, exist_ok=True)
            with hook_factory(prof_dir, [0]):
                res = run_bass_kernel_spmd(
                    nc, in_maps, core_ids=list(range(N_CORES))
                )
        except Exception as e:  # noqa: BLE001
            print("profiling failed, running untraced:", e)
            res = None
    if res is None:
        res = run_bass_kernel_spmd(nc, in_maps, core_ids=list(range(N_CORES)))
    LAST_EXEC_NS = res.exec_time_ns

    out = np.empty((B_FULL, CH, H, W), dtype=np.float32)
    for s in range(N_CORES):
        shard_view = out[s * B_SHARD:(s + 1) * B_SHARD].reshape(N_IMG, H, W)
        shard_view[PERM[:N_MAIN]] = res.results[s]["out"].astype(np.float32)
        shard_view[PERM[N_MAIN:]] = (
            res.results[s]["outs"]
            .reshape(H, N_SPEC, W)
            .transpose(1, 0, 2)
            .astype(np.float32)
        )
    return out
